# revision 96
# baseline (speedup 1.0000x reference)
"""Trainium2 Bass kernel for nn_CIFM_63780264345953.

Reference computation (per batch b of 8):
    S      = (Q @ K^T) * scale_param / sqrt(512)        [N, N]
    A      = softmax(S, axis=-1)
    R      = relu(A @ V)                                [N, D]
    C      = relu((V - R) @ W^T)                        [N, D]
    out    = a * R + b * C
Sharding: data-parallel over batch B=8 across the 8 NeuronCores.

Per-core kernel (N=2048, D=512):
  - Q, K cast fp32->fp8e4 in the DMA; PE-transposed (stride-2 fp8 PSUM out)
    into Q^T, K^T [d, n] fp8 layouts.
  - S^T tiles [m, n] via fp8 DoubleRow matmuls (K=256 per instruction),
    exp on ScalarE with bias -3.5 folded in (keeps e^s inside fp8e4's
    +-240 range even at the data's max score ~8.0; the constant cancels
    in softmax), fp8 output.
  - A@[V|1] via DoubleRow (V loaded again as fp8; ones column gives the
    softmax denominator in the same PSUM tile).
  - Query dim split in 4 quarters, software-pipelined: scores(q+1) and
    AV/C(q) interleave so ScalarE's exp stream hides behind PE work; the
    AV -> X^T -> C chain is lagged (XT one tile behind AV, C two behind)
    so the in-order PE queue never waits on DVE/Pool round trips.
  - C = relu((V-R) @ W^T) stays bf16 (fp8 would eat the error budget).
  - GpSimd cannot touch PSUM, so it gets only SBUF-SBUF ops (V-R
    subtract, final add) plus SWDGE desc-gen; DVE takes the PSUM-side
    element ops (relu-scale, X^T/C copies, relu(C)*b); ScalarE runs exp
    plus a few head packing copies (exp/relu/copy share one act table).
    Output stores on SP HWDGE. Scores exp'd in m-tile PAIRS (one wide
    ACT instruction per two S^T tiles) to halve ACT overhead; K^T/Q^T
    packing copies spread over DVE/ScalarE with a 4-deep transpose
    PSUM pool that closes after the head to hand its banks to AV/C.
"""

import math

import numpy as np

B, N_FULL, D_FULL = 8, 2048, 512
P = 128


def _build_bass(N, D, scale, a_val, b_val, reps=1):
    import concourse.tile as tile
    from concourse import bacc, mybir
    from concourse.masks import make_identity
    from contextlib import ExitStack

    f32 = mybir.dt.float32
    bf16 = mybir.dt.bfloat16
    fp8 = mybir.dt.float8e4
    NB = N // P          # seq blocks (16)
    DB = D // P          # feature blocks (4)
    QW = N // 4          # query-quarter width (512)
    EXP_BIAS = -3.5

    nc = bacc.Bacc(None)
    q = nc.declare_dram_parameter("q", [N, D], f32, isOutput=False)
    k = nc.declare_dram_parameter("k", [N, D], f32, isOutput=False)
    v = nc.declare_dram_parameter("v", [N, D], f32, isOutput=False)
    w = nc.declare_dram_parameter("w", [D, D], f32, isOutput=False)
    out = nc.declare_dram_parameter("out", [N, D], f32, isOutput=True)

    q3 = q.rearrange("(nb p) d -> p nb d", p=P)
    k3 = k.rearrange("(nb p) d -> p nb d", p=P)
    v3 = v.rearrange("(nb p) d -> p nb d", p=P)
    w3 = w.rearrange("(ob p) d -> p ob d", p=P)
    out3 = out.rearrange("(nb p) d -> p nb d", p=P)

    with ExitStack() as ctx:
        tc = ctx.enter_context(tile.TileContext(nc))

        persist = ctx.enter_context(tc.tile_pool(name="persist", bufs=1))
        qt = persist.tile([P, DB, N], fp8, tag="qt")      # Q^T [d, n]
        kt = persist.tile([P, DB, N], fp8, tag="kt")      # K^T [d, m]
        vhat = persist.tile([P, NB, 528], fp8, tag="vhat")  # V | ones | pad
        vbf = persist.tile([P, NB, D], bf16, tag="vbf")   # V bf16 (for V-R)
        wt = persist.tile([P, DB, D], bf16, tag="wt")     # W^T [d, o]
        exps = [
            persist.tile([P, NB, QW], fp8, tag=f"exps{h}", name=f"exps{h}")
            for h in range(4)
        ]
        ident8 = persist.tile([P, P], fp8, tag="ident8")
        # memset on DVE so Pool's affine_select is ready before the first
        # SWDGE desc-gen grabs the Pool engine
        nc.vector.memset(ident8, 0.0)
        make_identity(nc, ident8, nomemset=True)
        # bf16 identity for the bf16 transposes (compiler requires matching
        # dtypes): cast-copy on DVE, off Pool's desc-gen critical path
        ident = persist.tile([P, P], bf16, tag="ident")
        nc.vector.tensor_copy(out=ident, in_=ident8)
        # touch exp early so the ACT table loads during the DMA-bound head
        warm = persist.tile([P, 1], f32, tag="warm")
        nc.vector.memset(warm, 0.0)
        bias_t = persist.tile([P, 1], f32, tag="bias")
        nc.vector.memset(bias_t, EXP_BIAS)
        nc.scalar.activation(out=warm, in_=warm,
                             func=mybir.ActivationFunctionType.Exp)
        # softmax-denominator ones column (value 1.0 exactly in fp8)
        nc.vector.memset(vhat[:, :, 512:528], 1.0)

        conv = ctx.enter_context(tc.tile_pool(name="conv", bufs=1))

        # ---------------- Phase 1: load (cast in DMA) + transpose ---------
        # K/Q loads in chunks (first ones small so S^T/exp start early);
        # stride-2 fp8 PE transposes, packing copies round-robin DVE/Pool.
        # The tp pool holds 6 PSUM banks and is CLOSED after the head (via
        # close_head) so the AV/XT/C pools can use the banks.
        tpA_stack = ExitStack()
        psum_tp = tpA_stack.enter_context(
            tc.tile_pool(name="psum_tp", bufs=4, space="PSUM", side="right"))
        wu_ps = psum_tp.tile([P, P], f32, tag="tp", name="wu_ps")
        for _ in range(6):
            nc.tensor.matmul(wu_ps, ident8, ident8, start=True, stop=True)

        def stage(src3, b0, nb, tag):
            cv = conv.tile([P, nb, D], fp8, tag=tag, name="cv")
            nc.gpsimd.dma_start(out=cv, in_=src3[:, b0:b0 + nb, :])
            return cv

        def tp_chunk(cv, dstT, b0, nb, engs, pool=None):
            # transpose nb blocks into [d, n] fp8 layout; one PSUM->SBUF
            # packing copy per ds, engine per `engs` list
            for ds in range(DB):
                tp = (pool or psum_tp).tile([P, 8 * P, 2], fp8, tag="tp",
                                            name="tp")
                tps = tp[:, 0:nb * P, 0]        # element step 2
                for j in range(nb):
                    nc.tensor.transpose(
                        tps[:, j * P:(j + 1) * P],
                        cv[:, j, ds * P:(ds + 1) * P],
                        ident8,
                    )
                dst = dstT[:, ds, b0 * P:(b0 + nb) * P]
                e = engs[ds % len(engs)]
                if e == "d":
                    nc.vector.tensor_copy(out=dst, in_=tps)
                elif e == "a":
                    nc.scalar.copy(out=dst, in_=tps)
                else:
                    nc.gpsimd.tensor_copy(out=dst, in_=tps)

        # loads ordered by when their consumers run; desc-gen all on Pool
        kh0a = stage(k3, 0, 4, "kh0a")
        qh0a = stage(q3, 0, 4, "qh0a")
        kh0b = stage(k3, 4, 4, "kh0b")
        qh0b = stage(q3, 4, 4, "qh0b")
        kh1 = stage(k3, 8, 8, "kh1")
        qh1 = stage(q3, 8, 8, "qh1")
        nc.gpsimd.dma_start(out=vbf[:, 0:4, :], in_=v3[:, 0:4, :])
        cvw = conv.tile([P, DB, D], bf16, tag="convw")
        nc.gpsimd.dma_start(out=cvw, in_=w3)
        nc.gpsimd.dma_start(out=vhat[:, :, 0:512], in_=v3)
        nc.gpsimd.dma_start(out=vbf[:, 4:NB, :], in_=v3[:, 4:NB, :])

        # kh0/qh0 split DVE/ACT (ACT copies run before any exp -> one table
        # swap); kh1 in 4-block chunks on DVE (gates exp(q0) m8-15); qh1 on
        # Pool once desc-gen drains; W on DVE
        tp_chunk(kh0a, kt, 0, 4, ["d", "a", "d", "a"])
        tp_chunk(qh0a, qt, 0, 4, ["d", "a", "d", "a"])
        tp_chunk(kh0b, kt, 4, 4, ["d", "a", "d", "a"])
        tp_chunk(qh0b, qt, 4, 4, ["d", "a", "d", "a"])

        def head_tail():
            # emitted between S(q0) m0-7 and m8-15 by _compute_phases
            tp_chunk(kh1, kt, 8, 4, ["d"])
            tp_chunk(kh1[:, 4:8, :], kt, 12, 4, ["d"])

        def head_tail2():
            for ds in range(DB):
                tpw = psum_tp.tile([P, DB * P], bf16, tag="tp", name="tpw")
                for ob in range(DB):
                    nc.tensor.transpose(
                        tpw[:, ob * P:(ob + 1) * P],
                        cvw[:, ob, ds * P:(ds + 1) * P],
                        ident,
                    )
                nc.vector.tensor_copy(out=wt[:, ds, :], in_=tpw)

        def qh1_piece(i, pool):
            # one 4-block ds-group of qh1's transpose+copy, woven into the
            # AV/C loop (shares the xt pool's PSUM slot) so DVE/PE absorb
            # it in their slack
            ds, half = i % DB, i // DB
            b0 = 8 + 4 * half
            tp = pool.tile([P, 4 * P, 2], fp8, tag="xt", name="tpb")
            tps = tp[:, :, 0]
            for j in range(4):
                nc.tensor.transpose(
                    tps[:, j * P:(j + 1) * P],
                    qh1[:, 4 * half + j, ds * P:(ds + 1) * P],
                    ident8,
                )
            nc.vector.tensor_copy(
                out=qt[:, ds, b0 * P:(b0 + 4) * P], in_=tps)

        # ---------------- Phase 2+3: pipelined S^T/exp and AV/C ----------
        for _rep in range(reps):
            _compute_phases(
                nc, tc, mybir, qt, kt, vhat, vbf, wt, exps, ident, out3,
                N, D, NB, DB, QW, scale, a_val, b_val, bias_t, _rep,
                head_tail if _rep == 0 else None,
                head_tail2 if _rep == 0 else None,
                tpA_stack.close if _rep == 0 else None,
                qh1_piece if _rep == 0 else None,
            )

    nc.finalize()
    return nc


def _compute_phases(nc, tc, mybir, qt, kt, vhat, vbf, wt, exps, ident, out3,
                    N, D, NB, DB, QW, scale, a_val, b_val, bias_t, rep,
                    head_tail=None, head_tail2=None,
                    close_a=None, qh1_piece=None):
    from contextlib import ExitStack
    P = 128
    f32 = mybir.dt.float32
    bf16 = mybir.dt.bfloat16
    DR = mybir.MatmulPerfMode.DoubleRow
    NQT = NB // 4        # n-tiles per query quarter (4)

    with (
        tc.tile_pool(name=f"psum_st{rep}", bufs=2, space="PSUM") as psum_st,
        tc.tile_pool(name=f"ph3_{rep}", bufs=4) as ph3,
        tc.tile_pool(name=f"ph3b{rep}", bufs=5) as ph3b,
        ExitStack() as inner,
    ):
        state = {}

        def emit_s(qq, mp):
            # two m-tiles -> one wide exp instruction (halves ACT overhead)
            st = psum_st.tile([P, 2, QW], f32, tag="st", name="st")
            for i in range(2):
                m = 2 * mp + i
                for p in range(2):
                    nc.tensor.matmul(
                        st[:, i, :],
                        kt[:, 2 * p:2 * p + 2, m * P:(m + 1) * P],
                        qt[:, 2 * p:2 * p + 2, qq * QW:(qq + 1) * QW],
                        start=(p == 0),
                        stop=(p == 1),
                        perf_mode=DR,
                    )
            nc.scalar.activation(
                out=exps[qq][:, 2 * mp:2 * mp + 2, :],
                in_=st,
                func=mybir.ActivationFunctionType.Exp,
                scale=float(scale),
                bias=bias_t,
            )

        # S(q0) woven with the remaining head transposes, then free the
        # head's PSUM banks for the AV/XT/C pools
        for mp in range(NB // 4):
            emit_s(0, mp)
        if head_tail is not None:
            head_tail()
        for mp in range(NB // 4, NB // 2):
            emit_s(0, mp)
        if head_tail2 is not None:
            head_tail2()
        if close_a is not None:
            close_a()
        psum_av = inner.enter_context(
            tc.tile_pool(name=f"psum_av{rep}", bufs=1, space="PSUM"))
        psum_xt = inner.enter_context(
            tc.tile_pool(name=f"psum_xt{rep}", bufs=1, space="PSUM"))
        psum_c = inner.enter_context(
            tc.tile_pool(name=f"psum_c{rep}", bufs=1, space="PSUM"))

        if True:
            def emit_a(n):
                qq, cl = n // NQT, (n % NQT) * P
                av = psum_av.tile([P, 513], f32, tag="av", name="av")
                for p in range(8):
                    lhsT = exps[qq][:, 2 * p:2 * p + 2, cl:cl + P]
                    nc.tensor.matmul(
                        av[:, 512:513],
                        lhsT,
                        vhat[:, 2 * p:2 * p + 2, 512:513],
                        start=(p == 0), stop=(p == 7),
                        perf_mode=DR,
                    )
                    nc.tensor.matmul(
                        av[:, 0:512],
                        lhsT,
                        vhat[:, 2 * p:2 * p + 2, 0:512],
                        start=(p == 0), stop=(p == 7),
                        perf_mode=DR,
                    )
                recip = ph3b.tile([P, 1], f32, tag="recip", name="recip")
                nc.vector.reciprocal(recip, av[:, 512:513])
                # r = relu(av/denom) on ScalarE (relu shares exp's act table,
                # so no table swap; recip > 0 lets relu commute with scaling)
                r_t = ph3.tile([P, D], bf16, tag="r", name="r_t")
                nc.vector.tensor_scalar(
                    out=r_t, in0=av[:, 0:512],
                    scalar1=recip, scalar2=0.0,
                    op0=mybir.AluOpType.mult, op1=mybir.AluOpType.max,
                )
                # x = V - R on GpSimd (SBUF-only operands)
                x_t = ph3b.tile([P, D], bf16, tag="x", name="x_t")
                x_eng = nc.vector if n >= NB - 1 else nc.gpsimd
                x_eng.tensor_tensor(
                    out=x_t, in0=vbf[:, n, :], in1=r_t,
                    op=mybir.AluOpType.subtract,
                )
                state[n] = (r_t, x_t)

            def emit_b(n):
                _, x_t = state[n]
                xt_ps = psum_xt.tile([P, DB, P], bf16, tag="xt", name="xt_ps")
                for j in range(DB):
                    nc.tensor.transpose(
                        xt_ps[:, j, :], x_t[:, j * P:(j + 1) * P], ident
                    )
                xt_sb = ph3b.tile([P, DB, P], bf16, tag="xts", name="xt_sb")
                # drain region: ACT is idle once exp ends, so its copy
                # parallels DVE's r/cb instead of queueing behind them
                if n >= NB - 3:
                    nc.scalar.copy(out=xt_sb, in_=xt_ps)
                else:
                    nc.vector.tensor_copy(out=xt_sb, in_=xt_ps)
                state[n] = (state[n][0], xt_sb)

            def emit_c(n):
                r_t, xt_sb = state.pop(n)
                c_ps = psum_c.tile([P, D], f32, tag="c", name="c_ps")
                for ds in range(DB):
                    nc.tensor.matmul(
                        c_ps,
                        xt_sb[:, ds, :],
                        wt[:, ds, :],
                        start=(ds == 0),
                        stop=(ds == DB - 1),
                    )
                # cb = b * relu(C) on GpSimd (relu first, so any b is fine)
                cb_t = ph3b.tile([P, D], f32, tag="cb", name="cb_t")
                nc.vector.tensor_scalar(
                    out=cb_t, in0=c_ps,
                    scalar1=0.0, scalar2=float(b_val),
                    op0=mybir.AluOpType.max, op1=mybir.AluOpType.mult,
                )
                o_t = ph3.tile([P, D], f32, tag="o", name="o_t")
                o_eng = nc.vector if n >= NB - 1 else nc.gpsimd
                if a_val == 1.0:
                    o_eng.tensor_tensor(
                        out=o_t, in0=cb_t, in1=r_t, op=mybir.AluOpType.add
                    )
                else:
                    ra_t = ph3b.tile([P, D], f32, tag="ra", name="ra_t")
                    o_eng.tensor_scalar(
                        out=ra_t, in0=r_t,
                        scalar1=float(a_val), scalar2=None,
                        op0=mybir.AluOpType.mult,
                    )
                    o_eng.tensor_tensor(
                        out=o_t, in0=cb_t, in1=ra_t, op=mybir.AluOpType.add
                    )
                nc.sync.dma_start(out=out3[:, n, :], in_=o_t)

            LAGB, LAGC, NSW = 1, 2, 2
            spairs = [(c, mp) for c in (1, 2, 3) for mp in range(NB // 2)]
            for n in range(NB):
                if spairs:
                    emit_s(*spairs.pop(0))
                if qh1_piece is not None and n < 8:
                    qh1_piece(n, psum_xt)
                emit_a(n)
                for _ in range(NSW - 1):
                    if spairs:
                        emit_s(*spairs.pop(0))
                if n >= LAGB:
                    emit_b(n - LAGB)
                if n >= LAGC:
                    emit_c(n - LAGC)
            for n in range(NB - LAGB, NB):
                emit_b(n)
            for n in range(NB - LAGC, NB):
                emit_c(n)


def kernel(Q, K, V, W, scale_param, a, b):
    import sys
    if "/opt/trn_rl_repo" not in sys.path:
        sys.path.insert(0, "/opt/trn_rl_repo")
    from concourse.bass_utils import run_bass_kernel_spmd

    Q = np.ascontiguousarray(np.asarray(Q, dtype=np.float32))
    K = np.ascontiguousarray(np.asarray(K, dtype=np.float32))
    V = np.ascontiguousarray(np.asarray(V, dtype=np.float32))
    W = np.ascontiguousarray(np.asarray(W, dtype=np.float32))
    scale = float(np.asarray(scale_param).reshape(-1)[0]) / math.sqrt(D_FULL)
    a_val = float(np.asarray(a).reshape(-1)[0])
    b_val = float(np.asarray(b).reshape(-1)[0])

    nc = _build_bass(N_FULL, D_FULL, scale, a_val, b_val)
    in_maps = [
        {"q": Q[i], "k": K[i], "v": V[i], "w": W} for i in range(B)
    ]
    res = run_bass_kernel_spmd(nc, in_maps, list(range(B)))
    global LAST_RUN
    LAST_RUN = res
    out = np.stack([res.results[i]["out"] for i in range(B)])
    return out.astype(np.float32)


LAST_RUN = None



# revision 99
# speedup vs baseline: 1.0068x; 1.0068x over previous
"""Trainium2 Bass kernel for nn_CIFM_63780264345953.

Reference computation (per batch b of 8):
    S      = (Q @ K^T) * scale_param / sqrt(512)        [N, N]
    A      = softmax(S, axis=-1)
    R      = relu(A @ V)                                [N, D]
    C      = relu((V - R) @ W^T)                        [N, D]
    out    = a * R + b * C
Sharding: data-parallel over batch B=8 across the 8 NeuronCores.

Per-core kernel (N=2048, D=512):
  - Q, K cast fp32->fp8e4 in the DMA; PE-transposed (stride-2 fp8 PSUM out)
    into Q^T, K^T [d, n] fp8 layouts.
  - S^T tiles [m, n] via fp8 DoubleRow matmuls (K=256 per instruction),
    exp on ScalarE with bias -3.5 folded in (keeps e^s inside fp8e4's
    +-240 range even at the data's max score ~8.0; the constant cancels
    in softmax), fp8 output.
  - A@[V|1] via DoubleRow (V loaded again as fp8; ones column gives the
    softmax denominator in the same PSUM tile).
  - Query dim split in 4 quarters, software-pipelined: scores(q+1) and
    AV/C(q) interleave so ScalarE's exp stream hides behind PE work; the
    AV -> X^T -> C chain is lagged (XT one tile behind AV, C two behind)
    so the in-order PE queue never waits on DVE/Pool round trips.
  - C = relu((V-R) @ W^T) stays bf16 (fp8 would eat the error budget).
  - GpSimd cannot touch PSUM, so it gets only SBUF-SBUF ops (V-R
    subtract, final add) plus SWDGE desc-gen; DVE takes the PSUM-side
    element ops (relu-scale, X^T/C copies, relu(C)*b); ScalarE runs exp
    plus a few head packing copies (exp/relu/copy share one act table).
    Output stores on SP HWDGE. Scores exp'd in m-tile PAIRS (one wide
    ACT instruction per two S^T tiles) to halve ACT overhead; K^T/Q^T
    packing copies spread over DVE/ScalarE with a 4-deep transpose
    PSUM pool that closes after the head to hand its banks to AV/C.
"""

import math

import numpy as np

B, N_FULL, D_FULL = 8, 2048, 512
P = 128


def _build_bass(N, D, scale, a_val, b_val, reps=1):
    import concourse.tile as tile
    from concourse import bacc, mybir
    from concourse.masks import make_identity
    from contextlib import ExitStack

    f32 = mybir.dt.float32
    bf16 = mybir.dt.bfloat16
    fp8 = mybir.dt.float8e4
    NB = N // P          # seq blocks (16)
    DB = D // P          # feature blocks (4)
    QW = N // 4          # query-quarter width (512)
    EXP_BIAS = -3.5

    nc = bacc.Bacc(None)
    q = nc.declare_dram_parameter("q", [N, D], f32, isOutput=False)
    k = nc.declare_dram_parameter("k", [N, D], f32, isOutput=False)
    v = nc.declare_dram_parameter("v", [N, D], f32, isOutput=False)
    w = nc.declare_dram_parameter("w", [D, D], f32, isOutput=False)
    out = nc.declare_dram_parameter("out", [N, D], f32, isOutput=True)

    q3 = q.rearrange("(nb p) d -> p nb d", p=P)
    k3 = k.rearrange("(nb p) d -> p nb d", p=P)
    v3 = v.rearrange("(nb p) d -> p nb d", p=P)
    w3 = w.rearrange("(ob p) d -> p ob d", p=P)
    out3 = out.rearrange("(nb p) d -> p nb d", p=P)

    with ExitStack() as ctx:
        tc = ctx.enter_context(tile.TileContext(nc))

        persist = ctx.enter_context(tc.tile_pool(name="persist", bufs=1))
        qt = persist.tile([P, DB, N], fp8, tag="qt")      # Q^T [d, n]
        kt = persist.tile([P, DB, N], fp8, tag="kt")      # K^T [d, m]
        vhat = persist.tile([P, NB, 528], fp8, tag="vhat")  # V | ones | pad
        vbf = persist.tile([P, NB, D], bf16, tag="vbf")   # V bf16 (for V-R)
        wt = persist.tile([P, DB, D], bf16, tag="wt")     # W^T [d, o]
        exps = [
            persist.tile([P, NB, QW], fp8, tag=f"exps{h}", name=f"exps{h}")
            for h in range(4)
        ]
        ident8 = persist.tile([P, P], fp8, tag="ident8")
        # memset on DVE so Pool's affine_select is ready before the first
        # SWDGE desc-gen grabs the Pool engine
        nc.vector.memset(ident8, 0.0)
        make_identity(nc, ident8, nomemset=True)
        # bf16 identity for the bf16 transposes (compiler requires matching
        # dtypes): cast-copy on DVE, off Pool's desc-gen critical path
        ident = persist.tile([P, P], bf16, tag="ident")
        nc.vector.tensor_copy(out=ident, in_=ident8)
        # touch exp early so the ACT table loads during the DMA-bound head
        warm = persist.tile([P, 1], f32, tag="warm")
        nc.vector.memset(warm, 0.0)
        bias_t = persist.tile([P, 1], f32, tag="bias")
        nc.vector.memset(bias_t, EXP_BIAS)
        nc.scalar.activation(out=warm, in_=warm,
                             func=mybir.ActivationFunctionType.Exp)
        # softmax-denominator ones column (value 1.0 exactly in fp8)
        nc.vector.memset(vhat[:, :, 512:528], 1.0)

        conv = ctx.enter_context(tc.tile_pool(name="conv", bufs=1))

        # ---------------- Phase 1: load (cast in DMA) + transpose ---------
        # K/Q loads in chunks (first ones small so S^T/exp start early);
        # stride-2 fp8 PE transposes, packing copies round-robin DVE/Pool.
        # The tp pool holds 6 PSUM banks and is CLOSED after the head (via
        # close_head) so the AV/XT/C pools can use the banks.
        tpA_stack = ExitStack()
        psum_tp = tpA_stack.enter_context(
            tc.tile_pool(name="psum_tp", bufs=4, space="PSUM", side="right"))
        wu_ps = psum_tp.tile([P, P], f32, tag="tp", name="wu_ps")
        for _ in range(6):
            nc.tensor.matmul(wu_ps, ident8, ident8, start=True, stop=True)

        def stage(src3, b0, nb, tag):
            cv = conv.tile([P, nb, D], fp8, tag=tag, name="cv")
            nc.gpsimd.dma_start(out=cv, in_=src3[:, b0:b0 + nb, :])
            return cv

        def tp_chunk(cv, dstT, b0, nb, engs, pool=None):
            # transpose nb blocks into [d, n] fp8 layout; one PSUM->SBUF
            # packing copy per ds, engine per `engs` list
            for ds in range(DB):
                tp = (pool or psum_tp).tile([P, 8 * P, 2], fp8, tag="tp",
                                            name="tp")
                tps = tp[:, 0:nb * P, 0]        # element step 2
                for j in range(nb):
                    nc.tensor.transpose(
                        tps[:, j * P:(j + 1) * P],
                        cv[:, j, ds * P:(ds + 1) * P],
                        ident8,
                    )
                dst = dstT[:, ds, b0 * P:(b0 + nb) * P]
                e = engs[ds % len(engs)]
                if e == "d":
                    nc.vector.tensor_copy(out=dst, in_=tps)
                elif e == "a":
                    nc.scalar.copy(out=dst, in_=tps)
                else:
                    nc.gpsimd.tensor_copy(out=dst, in_=tps)

        # loads ordered by when their consumers run; desc-gen all on Pool
        kh0a = stage(k3, 0, 4, "kh0a")
        qh0a = stage(q3, 0, 4, "qh0a")
        kh0b = stage(k3, 4, 4, "kh0b")
        qh0b = stage(q3, 4, 4, "qh0b")
        kh1 = stage(k3, 8, 8, "kh1")
        qh1 = stage(q3, 8, 8, "qh1")
        nc.gpsimd.dma_start(out=vbf[:, 0:4, :], in_=v3[:, 0:4, :])
        cvw = conv.tile([P, DB, D], bf16, tag="convw")
        nc.gpsimd.dma_start(out=cvw, in_=w3)
        nc.gpsimd.dma_start(out=vhat[:, :, 0:512], in_=v3)
        nc.gpsimd.dma_start(out=vbf[:, 4:NB, :], in_=v3[:, 4:NB, :])

        # kh0/qh0 split DVE/ACT (ACT copies run before any exp -> one table
        # swap); kh1 in 4-block chunks on DVE (gates exp(q0) m8-15); qh1 on
        # Pool once desc-gen drains; W on DVE
        tp_chunk(kh0a, kt, 0, 4, ["d", "a", "d", "a"])
        tp_chunk(qh0a, qt, 0, 4, ["d", "a", "d", "a"])
        tp_chunk(kh0b, kt, 4, 4, ["d", "a", "d", "a"])
        tp_chunk(qh0b, qt, 4, 4, ["d", "a", "d", "a"])

        def head_tail():
            # emitted between S(q0) m0-7 and m8-15 by _compute_phases
            tp_chunk(kh1, kt, 8, 4, ["d"])
            tp_chunk(kh1[:, 4:8, :], kt, 12, 4, ["d"])

        def head_tail2():
            for ds in range(DB):
                tpw = psum_tp.tile([P, DB * P], bf16, tag="tp", name="tpw")
                for ob in range(DB):
                    nc.tensor.transpose(
                        tpw[:, ob * P:(ob + 1) * P],
                        cvw[:, ob, ds * P:(ds + 1) * P],
                        ident,
                    )
                nc.vector.tensor_copy(out=wt[:, ds, :], in_=tpw)

        def qh1_piece(i, pool):
            # one 4-block ds-group of qh1's transpose+copy, woven into the
            # AV/C loop (shares the xt pool's PSUM slot) so DVE/PE absorb
            # it in their slack
            ds, half = i % DB, i // DB
            b0 = 8 + 4 * half
            tp = pool.tile([P, 4 * P, 2], fp8, tag="xt", name="tpb")
            tps = tp[:, :, 0]
            for j in range(4):
                nc.tensor.transpose(
                    tps[:, j * P:(j + 1) * P],
                    qh1[:, 4 * half + j, ds * P:(ds + 1) * P],
                    ident8,
                )
            nc.vector.tensor_copy(
                out=qt[:, ds, b0 * P:(b0 + 4) * P], in_=tps)

        # ---------------- Phase 2+3: pipelined S^T/exp and AV/C ----------
        for _rep in range(reps):
            _compute_phases(
                nc, tc, mybir, qt, kt, vhat, vbf, wt, exps, ident, out3,
                N, D, NB, DB, QW, scale, a_val, b_val, bias_t, _rep,
                head_tail if _rep == 0 else None,
                head_tail2 if _rep == 0 else None,
                tpA_stack.close if _rep == 0 else None,
                qh1_piece if _rep == 0 else None,
            )

    nc.finalize()
    return nc


def _compute_phases(nc, tc, mybir, qt, kt, vhat, vbf, wt, exps, ident, out3,
                    N, D, NB, DB, QW, scale, a_val, b_val, bias_t, rep,
                    head_tail=None, head_tail2=None,
                    close_a=None, qh1_piece=None):
    from contextlib import ExitStack
    P = 128
    f32 = mybir.dt.float32
    bf16 = mybir.dt.bfloat16
    DR = mybir.MatmulPerfMode.DoubleRow
    NQT = NB // 4        # n-tiles per query quarter (4)

    with (
        tc.tile_pool(name=f"psum_st{rep}", bufs=2, space="PSUM") as psum_st,
        tc.tile_pool(name=f"ph3_{rep}", bufs=4) as ph3,
        tc.tile_pool(name=f"ph3b{rep}", bufs=5) as ph3b,
        ExitStack() as inner,
    ):
        state = {}

        def emit_s(qq, mp):
            # two m-tiles -> one wide exp instruction (halves ACT overhead)
            st = psum_st.tile([P, 2, QW], f32, tag="st", name="st")
            for i in range(2):
                m = 2 * mp + i
                for p in range(2):
                    nc.tensor.matmul(
                        st[:, i, :],
                        kt[:, 2 * p:2 * p + 2, m * P:(m + 1) * P],
                        qt[:, 2 * p:2 * p + 2, qq * QW:(qq + 1) * QW],
                        start=(p == 0),
                        stop=(p == 1),
                        perf_mode=DR,
                    )
            nc.scalar.activation(
                out=exps[qq][:, 2 * mp:2 * mp + 2, :],
                in_=st,
                func=mybir.ActivationFunctionType.Exp,
                scale=float(scale),
                bias=bias_t,
            )

        # S(q0) woven with the remaining head transposes, then free the
        # head's PSUM banks for the AV/XT/C pools
        for mp in range(NB // 4):
            emit_s(0, mp)
        if head_tail is not None:
            head_tail()
        for mp in range(NB // 4, NB // 2):
            emit_s(0, mp)
        if head_tail2 is not None:
            head_tail2()
        if close_a is not None:
            close_a()
        psum_av = inner.enter_context(
            tc.tile_pool(name=f"psum_av{rep}", bufs=1, space="PSUM"))
        psum_xt = inner.enter_context(
            tc.tile_pool(name=f"psum_xt{rep}", bufs=1, space="PSUM"))
        psum_c = inner.enter_context(
            tc.tile_pool(name=f"psum_c{rep}", bufs=1, space="PSUM"))

        if True:
            def emit_a(n):
                qq, cl = n // NQT, (n % NQT) * P
                av = psum_av.tile([P, 513], f32, tag="av", name="av")
                for p in range(8):
                    lhsT = exps[qq][:, 2 * p:2 * p + 2, cl:cl + P]
                    nc.tensor.matmul(
                        av[:, 512:513],
                        lhsT,
                        vhat[:, 2 * p:2 * p + 2, 512:513],
                        start=(p == 0), stop=(p == 7),
                        perf_mode=DR,
                    )
                    nc.tensor.matmul(
                        av[:, 0:512],
                        lhsT,
                        vhat[:, 2 * p:2 * p + 2, 0:512],
                        start=(p == 0), stop=(p == 7),
                        perf_mode=DR,
                    )
                recip = ph3b.tile([P, 1], f32, tag="recip", name="recip")
                nc.vector.reciprocal(recip, av[:, 512:513])
                # r = relu(av/denom) on ScalarE (relu shares exp's act table,
                # so no table swap; recip > 0 lets relu commute with scaling)
                r_t = ph3.tile([P, D], bf16, tag="r", name="r_t")
                nc.vector.tensor_scalar(
                    out=r_t, in0=av[:, 0:512],
                    scalar1=recip, scalar2=0.0,
                    op0=mybir.AluOpType.mult, op1=mybir.AluOpType.max,
                )
                # x = V - R on GpSimd (SBUF-only operands)
                x_t = ph3b.tile([P, D], bf16, tag="x", name="x_t")
                x_eng = nc.vector if n >= NB - 1 else nc.gpsimd
                x_eng.tensor_tensor(
                    out=x_t, in0=vbf[:, n, :], in1=r_t,
                    op=mybir.AluOpType.subtract,
                )
                state[n] = (r_t, x_t)

            def emit_b(n):
                _, x_t = state[n]
                xt_ps = psum_xt.tile([P, DB, P], bf16, tag="xt", name="xt_ps")
                for j in range(DB):
                    nc.tensor.transpose(
                        xt_ps[:, j, :], x_t[:, j * P:(j + 1) * P], ident
                    )
                xt_sb = ph3b.tile([P, DB, P], bf16, tag="xts", name="xt_sb")
                # drain region: ACT is idle once exp ends, so its copy
                # parallels DVE's r/cb instead of queueing behind them
                if n >= NB - 3:
                    nc.scalar.copy(out=xt_sb, in_=xt_ps)
                else:
                    nc.vector.tensor_copy(out=xt_sb, in_=xt_ps)
                state[n] = (state[n][0], xt_sb)

            def emit_c(n):
                r_t, xt_sb = state.pop(n)
                if n == NB - 1:
                    # the exp stream is done by the drain, so the final C
                    # borrows a score-PSUM tile: breaks the c(15)-waits-
                    # cb(14) serialization on the single c bank
                    c_ps = psum_st.tile([P, 2, QW], f32, tag="st",
                                        name="c15")[:, 0, :]
                else:
                    c_ps = psum_c.tile([P, D], f32, tag="c", name="c_ps")
                for ds in range(DB):
                    nc.tensor.matmul(
                        c_ps,
                        xt_sb[:, ds, :],
                        wt[:, ds, :],
                        start=(ds == 0),
                        stop=(ds == DB - 1),
                    )
                # cb = b * relu(C) on GpSimd (relu first, so any b is fine)
                cb_t = ph3b.tile([P, D], f32, tag="cb", name="cb_t")
                nc.vector.tensor_scalar(
                    out=cb_t, in0=c_ps,
                    scalar1=0.0, scalar2=float(b_val),
                    op0=mybir.AluOpType.max, op1=mybir.AluOpType.mult,
                )
                o_t = ph3.tile([P, D], f32, tag="o", name="o_t")
                o_eng = nc.vector if n >= NB - 1 else nc.gpsimd
                if a_val == 1.0:
                    o_eng.tensor_tensor(
                        out=o_t, in0=cb_t, in1=r_t, op=mybir.AluOpType.add
                    )
                else:
                    ra_t = ph3b.tile([P, D], f32, tag="ra", name="ra_t")
                    o_eng.tensor_scalar(
                        out=ra_t, in0=r_t,
                        scalar1=float(a_val), scalar2=None,
                        op0=mybir.AluOpType.mult,
                    )
                    o_eng.tensor_tensor(
                        out=o_t, in0=cb_t, in1=ra_t, op=mybir.AluOpType.add
                    )
                nc.sync.dma_start(out=out3[:, n, :], in_=o_t)

            LAGB, LAGC, NSW = 1, 2, 2
            spairs = [(c, mp) for c in (1, 2, 3) for mp in range(NB // 2)]
            for n in range(NB):
                if spairs:
                    emit_s(*spairs.pop(0))
                if qh1_piece is not None and n < 8:
                    qh1_piece(n, psum_xt)
                emit_a(n)
                for _ in range(NSW - 1):
                    if spairs:
                        emit_s(*spairs.pop(0))
                if n >= LAGB:
                    emit_b(n - LAGB)
                if n >= LAGC:
                    emit_c(n - LAGC)
            for n in range(NB - LAGB, NB):
                emit_b(n)
            for n in range(NB - LAGC, NB):
                emit_c(n)


def kernel(Q, K, V, W, scale_param, a, b):
    import sys
    if "/opt/trn_rl_repo" not in sys.path:
        sys.path.insert(0, "/opt/trn_rl_repo")
    from concourse.bass_utils import run_bass_kernel_spmd

    Q = np.ascontiguousarray(np.asarray(Q, dtype=np.float32))
    K = np.ascontiguousarray(np.asarray(K, dtype=np.float32))
    V = np.ascontiguousarray(np.asarray(V, dtype=np.float32))
    W = np.ascontiguousarray(np.asarray(W, dtype=np.float32))
    scale = float(np.asarray(scale_param).reshape(-1)[0]) / math.sqrt(D_FULL)
    a_val = float(np.asarray(a).reshape(-1)[0])
    b_val = float(np.asarray(b).reshape(-1)[0])

    nc = _build_bass(N_FULL, D_FULL, scale, a_val, b_val)
    in_maps = [
        {"q": Q[i], "k": K[i], "v": V[i], "w": W} for i in range(B)
    ]
    res = run_bass_kernel_spmd(nc, in_maps, list(range(B)))
    global LAST_RUN
    LAST_RUN = res
    out = np.stack([res.results[i]["out"] for i in range(B)])
    return out.astype(np.float32)


LAST_RUN = None



# revision 103
# speedup vs baseline: 1.0107x; 1.0039x over previous
"""Trainium2 Bass kernel for nn_CIFM_63780264345953.

Reference computation (per batch b of 8):
    S      = (Q @ K^T) * scale_param / sqrt(512)        [N, N]
    A      = softmax(S, axis=-1)
    R      = relu(A @ V)                                [N, D]
    C      = relu((V - R) @ W^T)                        [N, D]
    out    = a * R + b * C
Sharding: data-parallel over batch B=8 across the 8 NeuronCores.

Per-core kernel (N=2048, D=512):
  - Q, K cast fp32->fp8e4 in the DMA; PE-transposed (stride-2 fp8 PSUM out)
    into Q^T, K^T [d, n] fp8 layouts.
  - S^T tiles [m, n] via fp8 DoubleRow matmuls (K=256 per instruction),
    exp on ScalarE with bias -3.5 folded in (keeps e^s inside fp8e4's
    +-240 range even at the data's max score ~8.0; the constant cancels
    in softmax), fp8 output.
  - A@[V|1] via DoubleRow (V loaded again as fp8; ones column gives the
    softmax denominator in the same PSUM tile).
  - Query dim split in 4 quarters, software-pipelined: scores(q+1) and
    AV/C(q) interleave so ScalarE's exp stream hides behind PE work; the
    AV -> X^T -> C chain is lagged (XT one tile behind AV, C two behind)
    so the in-order PE queue never waits on DVE/Pool round trips.
  - C = relu((V-R) @ W^T) stays bf16 (fp8 would eat the error budget).
  - GpSimd cannot touch PSUM, so it gets only SBUF-SBUF ops (V-R
    subtract, final add) plus SWDGE desc-gen; DVE takes the PSUM-side
    element ops (relu-scale, X^T/C copies, relu(C)*b); ScalarE runs exp
    plus a few head packing copies (exp/relu/copy share one act table).
    Output stores on SP HWDGE. Scores exp'd in m-tile PAIRS (one wide
    ACT instruction per two S^T tiles) to halve ACT overhead; K^T/Q^T
    packing copies spread over DVE/ScalarE with a 4-deep transpose
    PSUM pool that closes after the head to hand its banks to AV/C.
"""

import math

import numpy as np

B, N_FULL, D_FULL = 8, 2048, 512
P = 128


def _build_bass(N, D, scale, a_val, b_val, reps=1):
    import concourse.tile as tile
    from concourse import bacc, mybir
    from concourse.masks import make_identity
    from contextlib import ExitStack

    f32 = mybir.dt.float32
    bf16 = mybir.dt.bfloat16
    fp8 = mybir.dt.float8e4
    NB = N // P          # seq blocks (16)
    DB = D // P          # feature blocks (4)
    QW = N // 4          # query-quarter width (512)
    EXP_BIAS = -3.5

    nc = bacc.Bacc(None)
    q = nc.declare_dram_parameter("q", [N, D], f32, isOutput=False)
    k = nc.declare_dram_parameter("k", [N, D], f32, isOutput=False)
    v = nc.declare_dram_parameter("v", [N, D], f32, isOutput=False)
    w = nc.declare_dram_parameter("w", [D, D], f32, isOutput=False)
    out = nc.declare_dram_parameter("out", [N, D], f32, isOutput=True)

    q3 = q.rearrange("(nb p) d -> p nb d", p=P)
    k3 = k.rearrange("(nb p) d -> p nb d", p=P)
    v3 = v.rearrange("(nb p) d -> p nb d", p=P)
    w3 = w.rearrange("(ob p) d -> p ob d", p=P)
    out3 = out.rearrange("(nb p) d -> p nb d", p=P)

    with ExitStack() as ctx:
        tc = ctx.enter_context(tile.TileContext(nc))

        persist = ctx.enter_context(tc.tile_pool(name="persist", bufs=1))
        qt = persist.tile([P, DB, N], fp8, tag="qt")      # Q^T [d, n]
        kt = persist.tile([P, DB, N], fp8, tag="kt")      # K^T [d, m]
        vhat = persist.tile([P, NB, 528], fp8, tag="vhat")  # V | ones | pad
        vbf = persist.tile([P, NB, D], bf16, tag="vbf")   # V bf16 (for V-R)
        wt = persist.tile([P, DB, D], bf16, tag="wt")     # W^T [d, o]
        exps = [
            persist.tile([P, NB, QW], fp8, tag=f"exps{h}", name=f"exps{h}")
            for h in range(4)
        ]
        ident8 = persist.tile([P, P], fp8, tag="ident8")
        # memset on DVE so Pool's affine_select is ready before the first
        # SWDGE desc-gen grabs the Pool engine
        nc.vector.memset(ident8, 0.0)
        make_identity(nc, ident8, nomemset=True)
        # bf16 identity for the bf16 transposes (compiler requires matching
        # dtypes): cast-copy on DVE, off Pool's desc-gen critical path
        ident = persist.tile([P, P], bf16, tag="ident")
        nc.vector.tensor_copy(out=ident, in_=ident8)
        # touch exp early so the ACT table loads during the DMA-bound head
        warm = persist.tile([P, 1], f32, tag="warm")
        nc.vector.memset(warm, 0.0)
        bias_t = persist.tile([P, 1], f32, tag="bias")
        nc.vector.memset(bias_t, EXP_BIAS)
        nc.scalar.activation(out=warm, in_=warm,
                             func=mybir.ActivationFunctionType.Exp)
        # softmax-denominator ones column (value 1.0 exactly in fp8)
        nc.vector.memset(vhat[:, :, 512:528], 1.0)

        conv = ctx.enter_context(tc.tile_pool(name="conv", bufs=1))

        # ---------------- Phase 1: load (cast in DMA) + transpose ---------
        # K/Q loads in chunks (first ones small so S^T/exp start early);
        # stride-2 fp8 PE transposes, packing copies round-robin DVE/Pool.
        # The tp pool holds 6 PSUM banks and is CLOSED after the head (via
        # close_head) so the AV/XT/C pools can use the banks.
        tpA_stack = ExitStack()
        psum_tp = tpA_stack.enter_context(
            tc.tile_pool(name="psum_tp", bufs=4, space="PSUM", side="right"))
        wu_ps = psum_tp.tile([P, P], f32, tag="tp", name="wu_ps")
        for _ in range(6):
            nc.tensor.matmul(wu_ps, ident8, ident8, start=True, stop=True)

        def stage(src3, b0, nb, tag):
            cv = conv.tile([P, nb, D], fp8, tag=tag, name="cv")
            nc.gpsimd.dma_start(out=cv, in_=src3[:, b0:b0 + nb, :])
            return cv

        def tp_chunk(cv, dstT, b0, nb, engs, pool=None):
            # transpose nb blocks into [d, n] fp8 layout; one PSUM->SBUF
            # packing copy per ds, engine per `engs` list
            for ds in range(DB):
                tp = (pool or psum_tp).tile([P, 8 * P, 2], fp8, tag="tp",
                                            name="tp")
                tps = tp[:, 0:nb * P, 0]        # element step 2
                for j in range(nb):
                    nc.tensor.transpose(
                        tps[:, j * P:(j + 1) * P],
                        cv[:, j, ds * P:(ds + 1) * P],
                        ident8,
                    )
                dst = dstT[:, ds, b0 * P:(b0 + nb) * P]
                e = engs[ds % len(engs)]
                if e == "d":
                    nc.vector.tensor_copy(out=dst, in_=tps)
                elif e == "a":
                    nc.scalar.copy(out=dst, in_=tps)
                else:
                    nc.gpsimd.tensor_copy(out=dst, in_=tps)

        # loads ordered by when their consumers run; desc-gen all on Pool
        kh0a = stage(k3, 0, 4, "kh0a")
        qh0a = stage(q3, 0, 4, "qh0a")
        kh0b = stage(k3, 4, 4, "kh0b")
        qh0b = stage(q3, 4, 4, "qh0b")
        kh1 = stage(k3, 8, 8, "kh1")
        qh1 = stage(q3, 8, 8, "qh1")
        nc.gpsimd.dma_start(out=vbf[:, 0:4, :], in_=v3[:, 0:4, :])
        cvw = conv.tile([P, DB, D], bf16, tag="convw")
        nc.gpsimd.dma_start(out=cvw, in_=w3)
        nc.gpsimd.dma_start(out=vhat[:, :, 0:512], in_=v3)
        nc.gpsimd.dma_start(out=vbf[:, 4:NB, :], in_=v3[:, 4:NB, :])

        # kh0/qh0 split DVE/ACT (ACT copies run before any exp -> one table
        # swap); kh1 in 4-block chunks on DVE (gates exp(q0) m8-15); qh1 on
        # Pool once desc-gen drains; W on DVE
        tp_chunk(kh0a, kt, 0, 4, ["d", "a", "d", "a"])
        tp_chunk(qh0a, qt, 0, 4, ["d", "a", "d", "a"])
        tp_chunk(kh0b, kt, 4, 4, ["d", "a", "d", "a"])
        tp_chunk(qh0b, qt, 4, 4, ["d", "a", "d", "a"])

        def head_tail():
            # emitted between S(q0) m0-7 and m8-15 by _compute_phases
            tp_chunk(kh1, kt, 8, 4, ["d"])
            tp_chunk(kh1[:, 4:8, :], kt, 12, 4, ["d"])

        def head_tail2():
            for ds in range(DB):
                tpw = psum_tp.tile([P, DB * P], bf16, tag="tp", name="tpw")
                for ob in range(DB):
                    nc.tensor.transpose(
                        tpw[:, ob * P:(ob + 1) * P],
                        cvw[:, ob, ds * P:(ds + 1) * P],
                        ident,
                    )
                nc.vector.tensor_copy(out=wt[:, ds, :], in_=tpw)

        def qh1_piece(i, pool):
            # one 4-block ds-group of qh1's transpose+copy, woven into the
            # AV/C loop (shares the xt pool's PSUM slot) so DVE/PE absorb
            # it in their slack
            ds, half = i % DB, i // DB
            b0 = 8 + 4 * half
            tp = pool.tile([P, 4 * P, 2], fp8, tag="xt", name="tpb")
            tps = tp[:, :, 0]
            for j in range(4):
                nc.tensor.transpose(
                    tps[:, j * P:(j + 1) * P],
                    qh1[:, 4 * half + j, ds * P:(ds + 1) * P],
                    ident8,
                )
            nc.vector.tensor_copy(
                out=qt[:, ds, b0 * P:(b0 + 4) * P], in_=tps)

        # ---------------- Phase 2+3: pipelined S^T/exp and AV/C ----------
        for _rep in range(reps):
            _compute_phases(
                nc, tc, mybir, qt, kt, vhat, vbf, wt, exps, ident, out3,
                N, D, NB, DB, QW, scale, a_val, b_val, bias_t, _rep,
                head_tail if _rep == 0 else None,
                head_tail2 if _rep == 0 else None,
                tpA_stack.close if _rep == 0 else None,
                qh1_piece if _rep == 0 else None,
            )

    nc.finalize()
    return nc


def _compute_phases(nc, tc, mybir, qt, kt, vhat, vbf, wt, exps, ident, out3,
                    N, D, NB, DB, QW, scale, a_val, b_val, bias_t, rep,
                    head_tail=None, head_tail2=None,
                    close_a=None, qh1_piece=None):
    from contextlib import ExitStack
    P = 128
    f32 = mybir.dt.float32
    bf16 = mybir.dt.bfloat16
    DR = mybir.MatmulPerfMode.DoubleRow
    NQT = NB // 4        # n-tiles per query quarter (4)

    with (
        tc.tile_pool(name=f"psum_st{rep}", bufs=2, space="PSUM") as psum_st,
        tc.tile_pool(name=f"ph3_{rep}", bufs=4) as ph3,
        tc.tile_pool(name=f"ph3b{rep}", bufs=5) as ph3b,
        ExitStack() as inner,
    ):
        state = {}

        def emit_s(qq, mp):
            # two m-tiles -> one wide exp instruction (halves ACT overhead)
            st = psum_st.tile([P, 2, QW], f32, tag="st", name="st")
            for i in range(2):
                m = 2 * mp + i
                for p in range(2):
                    nc.tensor.matmul(
                        st[:, i, :],
                        kt[:, 2 * p:2 * p + 2, m * P:(m + 1) * P],
                        qt[:, 2 * p:2 * p + 2, qq * QW:(qq + 1) * QW],
                        start=(p == 0),
                        stop=(p == 1),
                        perf_mode=DR,
                    )
            nc.scalar.activation(
                out=exps[qq][:, 2 * mp:2 * mp + 2, :],
                in_=st,
                func=mybir.ActivationFunctionType.Exp,
                scale=float(scale),
                bias=bias_t,
            )

        # S(q0) woven with the remaining head transposes, then free the
        # head's PSUM banks for the AV/XT/C pools
        for mp in range(NB // 4):
            emit_s(0, mp)
        if head_tail is not None:
            head_tail()
        for mp in range(NB // 4, NB // 2):
            emit_s(0, mp)
        if head_tail2 is not None:
            head_tail2()
        if close_a is not None:
            close_a()
        psum_av = inner.enter_context(
            tc.tile_pool(name=f"psum_av{rep}", bufs=1, space="PSUM"))
        psum_xt = inner.enter_context(
            tc.tile_pool(name=f"psum_xt{rep}", bufs=1, space="PSUM"))
        psum_c = inner.enter_context(
            tc.tile_pool(name=f"psum_c{rep}", bufs=1, space="PSUM"))

        if True:
            def emit_a(n):
                qq, cl = n // NQT, (n % NQT) * P
                av = psum_av.tile([P, 513], f32, tag="av", name="av")
                for p in range(8):
                    lhsT = exps[qq][:, 2 * p:2 * p + 2, cl:cl + P]
                    nc.tensor.matmul(
                        av[:, 512:513],
                        lhsT,
                        vhat[:, 2 * p:2 * p + 2, 512:513],
                        start=(p == 0), stop=(p == 7),
                        perf_mode=DR,
                    )
                    nc.tensor.matmul(
                        av[:, 0:512],
                        lhsT,
                        vhat[:, 2 * p:2 * p + 2, 0:512],
                        start=(p == 0), stop=(p == 7),
                        perf_mode=DR,
                    )
                recip = ph3b.tile([P, 1], f32, tag="recip", name="recip")
                nc.vector.reciprocal(recip, av[:, 512:513])
                # r = relu(av/denom) on ScalarE (relu shares exp's act table,
                # so no table swap; recip > 0 lets relu commute with scaling)
                r_t = ph3.tile([P, D], bf16, tag="r", name="r_t")
                nc.vector.tensor_scalar(
                    out=r_t, in0=av[:, 0:512],
                    scalar1=recip, scalar2=0.0,
                    op0=mybir.AluOpType.mult, op1=mybir.AluOpType.max,
                )
                # x = V - R on GpSimd (SBUF-only operands)
                x_t = ph3b.tile([P, D], bf16, tag="x", name="x_t")
                x_eng = nc.vector if n >= NB - 1 else nc.gpsimd
                x_eng.tensor_tensor(
                    out=x_t, in0=vbf[:, n, :], in1=r_t,
                    op=mybir.AluOpType.subtract,
                )
                state[n] = (r_t, x_t)

            def emit_b(n):
                _, x_t = state[n]
                xt_ps = psum_xt.tile([P, DB, P], bf16, tag="xt", name="xt_ps")
                for j in range(DB):
                    nc.tensor.transpose(
                        xt_ps[:, j, :], x_t[:, j * P:(j + 1) * P], ident
                    )
                xt_sb = ph3b.tile([P, DB, P], bf16, tag="xts", name="xt_sb")
                # drain region: ACT is idle once exp ends, so its copy
                # parallels DVE's r/cb instead of queueing behind them
                if n >= NB - 2:
                    nc.scalar.copy(out=xt_sb, in_=xt_ps)
                else:
                    nc.vector.tensor_copy(out=xt_sb, in_=xt_ps)
                state[n] = (state[n][0], xt_sb)

            def emit_c(n):
                r_t, xt_sb = state.pop(n)
                if n >= NB - 2:
                    # the exp stream is done by the drain, so the last two
                    # C tiles borrow score-PSUM tiles: breaks the
                    # c(n)-waits-cb(n-1) serialization on the single c bank
                    c_ps = psum_st.tile([P, 2, QW], f32, tag="st",
                                        name="c15")[:, 0, :]
                else:
                    c_ps = psum_c.tile([P, D], f32, tag="c", name="c_ps")
                for ds in range(DB):
                    nc.tensor.matmul(
                        c_ps,
                        xt_sb[:, ds, :],
                        wt[:, ds, :],
                        start=(ds == 0),
                        stop=(ds == DB - 1),
                    )
                # cb = b * relu(C) on GpSimd (relu first, so any b is fine)
                cb_t = ph3b.tile([P, D], f32, tag="cb", name="cb_t")
                nc.vector.tensor_scalar(
                    out=cb_t, in0=c_ps,
                    scalar1=0.0, scalar2=float(b_val),
                    op0=mybir.AluOpType.max, op1=mybir.AluOpType.mult,
                )
                o_t = ph3.tile([P, D], f32, tag="o", name="o_t")
                o_eng = nc.vector if n >= NB - 1 else nc.gpsimd
                if a_val == 1.0:
                    o_eng.tensor_tensor(
                        out=o_t, in0=cb_t, in1=r_t, op=mybir.AluOpType.add
                    )
                else:
                    ra_t = ph3b.tile([P, D], f32, tag="ra", name="ra_t")
                    o_eng.tensor_scalar(
                        out=ra_t, in0=r_t,
                        scalar1=float(a_val), scalar2=None,
                        op0=mybir.AluOpType.mult,
                    )
                    o_eng.tensor_tensor(
                        out=o_t, in0=cb_t, in1=ra_t, op=mybir.AluOpType.add
                    )
                nc.sync.dma_start(out=out3[:, n, :], in_=o_t)

            LAGB, LAGC, NSW = 1, 2, 2
            spairs = [(c, mp) for c in (1, 2, 3) for mp in range(NB // 2)]
            for n in range(NB):
                if spairs:
                    emit_s(*spairs.pop(0))
                if qh1_piece is not None and n < 8:
                    qh1_piece(n, psum_xt)
                emit_a(n)
                for _ in range(NSW - 1):
                    if spairs:
                        emit_s(*spairs.pop(0))
                if n >= LAGB:
                    emit_b(n - LAGB)
                if n >= LAGC:
                    emit_c(n - LAGC)
            for n in range(NB - LAGB, NB):
                emit_b(n)
            for n in range(NB - LAGC, NB):
                emit_c(n)


def kernel(Q, K, V, W, scale_param, a, b):
    import sys
    if "/opt/trn_rl_repo" not in sys.path:
        sys.path.insert(0, "/opt/trn_rl_repo")
    from concourse.bass_utils import run_bass_kernel_spmd

    Q = np.ascontiguousarray(np.asarray(Q, dtype=np.float32))
    K = np.ascontiguousarray(np.asarray(K, dtype=np.float32))
    V = np.ascontiguousarray(np.asarray(V, dtype=np.float32))
    W = np.ascontiguousarray(np.asarray(W, dtype=np.float32))
    scale = float(np.asarray(scale_param).reshape(-1)[0]) / math.sqrt(D_FULL)
    a_val = float(np.asarray(a).reshape(-1)[0])
    b_val = float(np.asarray(b).reshape(-1)[0])

    nc = _build_bass(N_FULL, D_FULL, scale, a_val, b_val)
    in_maps = [
        {"q": Q[i], "k": K[i], "v": V[i], "w": W} for i in range(B)
    ]
    res = run_bass_kernel_spmd(nc, in_maps, list(range(B)))
    global LAST_RUN
    LAST_RUN = res
    out = np.stack([res.results[i]["out"] for i in range(B)])
    return out.astype(np.float32)


LAST_RUN = None



# revision 104
# speedup vs baseline: 1.0119x; 1.0012x over previous
"""Trainium2 Bass kernel for nn_CIFM_63780264345953.

Reference computation (per batch b of 8):
    S      = (Q @ K^T) * scale_param / sqrt(512)        [N, N]
    A      = softmax(S, axis=-1)
    R      = relu(A @ V)                                [N, D]
    C      = relu((V - R) @ W^T)                        [N, D]
    out    = a * R + b * C
Sharding: data-parallel over batch B=8 across the 8 NeuronCores.

Per-core kernel (N=2048, D=512):
  - Q, K cast fp32->fp8e4 in the DMA; PE-transposed (stride-2 fp8 PSUM out)
    into Q^T, K^T [d, n] fp8 layouts.
  - S^T tiles [m, n] via fp8 DoubleRow matmuls (K=256 per instruction),
    exp on ScalarE with bias -3.5 folded in (keeps e^s inside fp8e4's
    +-240 range even at the data's max score ~8.0; the constant cancels
    in softmax), fp8 output.
  - A@[V|1] via DoubleRow (V loaded again as fp8; ones column gives the
    softmax denominator in the same PSUM tile).
  - Query dim split in 4 quarters, software-pipelined: scores(q+1) and
    AV/C(q) interleave so ScalarE's exp stream hides behind PE work; the
    AV -> X^T -> C chain is lagged (XT one tile behind AV, C two behind)
    so the in-order PE queue never waits on DVE/Pool round trips.
  - C = relu((V-R) @ W^T) stays bf16 (fp8 would eat the error budget).
  - GpSimd cannot touch PSUM, so it gets only SBUF-SBUF ops (V-R
    subtract, final add) plus SWDGE desc-gen; DVE takes the PSUM-side
    element ops (relu-scale, X^T/C copies, relu(C)*b); ScalarE runs exp
    plus a few head packing copies (exp/relu/copy share one act table).
    Output stores on SP HWDGE. Scores exp'd in m-tile PAIRS (one wide
    ACT instruction per two S^T tiles) to halve ACT overhead; K^T/Q^T
    packing copies spread over DVE/ScalarE with a 4-deep transpose
    PSUM pool that closes after the head to hand its banks to AV/C.
"""

import math

import numpy as np

B, N_FULL, D_FULL = 8, 2048, 512
P = 128


def _build_bass(N, D, scale, a_val, b_val, reps=1):
    import concourse.tile as tile
    from concourse import bacc, mybir
    from concourse.masks import make_identity
    from contextlib import ExitStack

    f32 = mybir.dt.float32
    bf16 = mybir.dt.bfloat16
    fp8 = mybir.dt.float8e4
    NB = N // P          # seq blocks (16)
    DB = D // P          # feature blocks (4)
    QW = N // 4          # query-quarter width (512)
    EXP_BIAS = -3.5

    nc = bacc.Bacc(None)
    q = nc.declare_dram_parameter("q", [N, D], f32, isOutput=False)
    k = nc.declare_dram_parameter("k", [N, D], f32, isOutput=False)
    v = nc.declare_dram_parameter("v", [N, D], f32, isOutput=False)
    w = nc.declare_dram_parameter("w", [D, D], f32, isOutput=False)
    out = nc.declare_dram_parameter("out", [N, D], f32, isOutput=True)

    q3 = q.rearrange("(nb p) d -> p nb d", p=P)
    k3 = k.rearrange("(nb p) d -> p nb d", p=P)
    v3 = v.rearrange("(nb p) d -> p nb d", p=P)
    w3 = w.rearrange("(ob p) d -> p ob d", p=P)
    out3 = out.rearrange("(nb p) d -> p nb d", p=P)

    with ExitStack() as ctx:
        tc = ctx.enter_context(tile.TileContext(nc))

        persist = ctx.enter_context(tc.tile_pool(name="persist", bufs=1))
        qt = persist.tile([P, DB, N], fp8, tag="qt")      # Q^T [d, n]
        kt = persist.tile([P, DB, N], fp8, tag="kt")      # K^T [d, m]
        vhat = persist.tile([P, NB, 528], fp8, tag="vhat")  # V | ones | pad
        vbf = persist.tile([P, NB, D], bf16, tag="vbf")   # V bf16 (for V-R)
        wt = persist.tile([P, DB, D], bf16, tag="wt")     # W^T [d, o]
        exps = [
            persist.tile([P, NB, QW], fp8, tag=f"exps{h}", name=f"exps{h}")
            for h in range(4)
        ]
        ident8 = persist.tile([P, P], fp8, tag="ident8")
        # memset on DVE so Pool's affine_select is ready before the first
        # SWDGE desc-gen grabs the Pool engine
        nc.vector.memset(ident8, 0.0)
        make_identity(nc, ident8, nomemset=True)
        # bf16 identity for the bf16 transposes (compiler requires matching
        # dtypes): cast-copy on DVE, off Pool's desc-gen critical path
        ident = persist.tile([P, P], bf16, tag="ident")
        nc.vector.tensor_copy(out=ident, in_=ident8)
        # touch exp early so the ACT table loads during the DMA-bound head
        warm = persist.tile([P, 1], f32, tag="warm")
        nc.vector.memset(warm, 0.0)
        bias_t = persist.tile([P, 1], f32, tag="bias")
        nc.vector.memset(bias_t, EXP_BIAS)
        nc.scalar.activation(out=warm, in_=warm,
                             func=mybir.ActivationFunctionType.Exp)
        # softmax-denominator ones column (value 1.0 exactly in fp8)
        nc.vector.memset(vhat[:, :, 512:528], 1.0)

        conv = ctx.enter_context(tc.tile_pool(name="conv", bufs=1))

        # ---------------- Phase 1: load (cast in DMA) + transpose ---------
        # K/Q loads in chunks (first ones small so S^T/exp start early);
        # stride-2 fp8 PE transposes, packing copies round-robin DVE/Pool.
        # The tp pool holds 6 PSUM banks and is CLOSED after the head (via
        # close_head) so the AV/XT/C pools can use the banks.
        tpA_stack = ExitStack()
        psum_tp = tpA_stack.enter_context(
            tc.tile_pool(name="psum_tp", bufs=4, space="PSUM", side="right"))
        wu_ps = psum_tp.tile([P, P], f32, tag="tp", name="wu_ps")
        for _ in range(6):
            nc.tensor.matmul(wu_ps, ident8, ident8, start=True, stop=True)

        def stage(src3, b0, nb, tag):
            cv = conv.tile([P, nb, D], fp8, tag=tag, name="cv")
            nc.gpsimd.dma_start(out=cv, in_=src3[:, b0:b0 + nb, :])
            return cv

        def tp_chunk(cv, dstT, b0, nb, engs, pool=None):
            # transpose nb blocks into [d, n] fp8 layout; one PSUM->SBUF
            # packing copy per ds, engine per `engs` list
            for ds in range(DB):
                tp = (pool or psum_tp).tile([P, 8 * P, 2], fp8, tag="tp",
                                            name="tp")
                tps = tp[:, 0:nb * P, 0]        # element step 2
                for j in range(nb):
                    nc.tensor.transpose(
                        tps[:, j * P:(j + 1) * P],
                        cv[:, j, ds * P:(ds + 1) * P],
                        ident8,
                    )
                dst = dstT[:, ds, b0 * P:(b0 + nb) * P]
                e = engs[ds % len(engs)]
                if e == "d":
                    nc.vector.tensor_copy(out=dst, in_=tps)
                elif e == "a":
                    nc.scalar.copy(out=dst, in_=tps)
                else:
                    nc.gpsimd.tensor_copy(out=dst, in_=tps)

        # loads ordered by when their consumers run; desc-gen all on Pool
        kh0a = stage(k3, 0, 4, "kh0a")
        qh0a = stage(q3, 0, 4, "qh0a")
        kh0b = stage(k3, 4, 4, "kh0b")
        qh0b = stage(q3, 4, 4, "qh0b")
        kh1 = stage(k3, 8, 8, "kh1")
        qh1 = stage(q3, 8, 8, "qh1")
        nc.gpsimd.dma_start(out=vbf[:, 0:4, :], in_=v3[:, 0:4, :])
        cvw = conv.tile([P, DB, D], bf16, tag="convw")
        nc.gpsimd.dma_start(out=cvw, in_=w3)
        nc.gpsimd.dma_start(out=vhat[:, :, 0:512], in_=v3)
        nc.gpsimd.dma_start(out=vbf[:, 4:NB, :], in_=v3[:, 4:NB, :])

        # kh0/qh0 split DVE/ACT (ACT copies run before any exp -> one table
        # swap); kh1 in 4-block chunks on DVE (gates exp(q0) m8-15); qh1 on
        # Pool once desc-gen drains; W on DVE
        tp_chunk(kh0a, kt, 0, 4, ["d", "a", "d", "a"])
        tp_chunk(qh0a, qt, 0, 4, ["d", "a", "d", "a"])
        tp_chunk(kh0b, kt, 4, 4, ["d", "a", "d", "a"])
        tp_chunk(qh0b, qt, 4, 4, ["d", "a", "d", "a"])

        def head_tail():
            # emitted between S(q0) m0-7 and m8-15 by _compute_phases
            tp_chunk(kh1, kt, 8, 4, ["d"])
            tp_chunk(kh1[:, 4:8, :], kt, 12, 4, ["d"])

        def head_tail2():
            for ds in range(DB):
                tpw = psum_tp.tile([P, DB * P], bf16, tag="tp", name="tpw")
                for ob in range(DB):
                    nc.tensor.transpose(
                        tpw[:, ob * P:(ob + 1) * P],
                        cvw[:, ob, ds * P:(ds + 1) * P],
                        ident,
                    )
                nc.vector.tensor_copy(out=wt[:, ds, :], in_=tpw)

        def qh1_piece(i, pool):
            # one 4-block ds-group of qh1's transpose+copy, woven into the
            # AV/C loop (shares the xt pool's PSUM slot) so DVE/PE absorb
            # it in their slack
            ds, half = i % DB, i // DB
            b0 = 8 + 4 * half
            tp = pool.tile([P, 4 * P, 2], fp8, tag="xt", name="tpb")
            tps = tp[:, :, 0]
            for j in range(4):
                nc.tensor.transpose(
                    tps[:, j * P:(j + 1) * P],
                    qh1[:, 4 * half + j, ds * P:(ds + 1) * P],
                    ident8,
                )
            nc.vector.tensor_copy(
                out=qt[:, ds, b0 * P:(b0 + 4) * P], in_=tps)

        # ---------------- Phase 2+3: pipelined S^T/exp and AV/C ----------
        for _rep in range(reps):
            _compute_phases(
                nc, tc, mybir, qt, kt, vhat, vbf, wt, exps, ident, out3,
                N, D, NB, DB, QW, scale, a_val, b_val, bias_t, _rep,
                head_tail if _rep == 0 else None,
                head_tail2 if _rep == 0 else None,
                tpA_stack.close if _rep == 0 else None,
                qh1_piece if _rep == 0 else None,
            )

    nc.finalize()
    return nc


def _compute_phases(nc, tc, mybir, qt, kt, vhat, vbf, wt, exps, ident, out3,
                    N, D, NB, DB, QW, scale, a_val, b_val, bias_t, rep,
                    head_tail=None, head_tail2=None,
                    close_a=None, qh1_piece=None):
    from contextlib import ExitStack
    P = 128
    f32 = mybir.dt.float32
    bf16 = mybir.dt.bfloat16
    DR = mybir.MatmulPerfMode.DoubleRow
    NQT = NB // 4        # n-tiles per query quarter (4)

    with (
        tc.tile_pool(name=f"psum_st{rep}", bufs=2, space="PSUM") as psum_st,
        tc.tile_pool(name=f"ph3_{rep}", bufs=4) as ph3,
        tc.tile_pool(name=f"ph3b{rep}", bufs=5) as ph3b,
        ExitStack() as inner,
    ):
        state = {}

        def emit_s(qq, mp):
            # two m-tiles -> one wide exp instruction (halves ACT overhead)
            st = psum_st.tile([P, 2, QW], f32, tag="st", name="st")
            for i in range(2):
                m = 2 * mp + i
                for p in range(2):
                    nc.tensor.matmul(
                        st[:, i, :],
                        kt[:, 2 * p:2 * p + 2, m * P:(m + 1) * P],
                        qt[:, 2 * p:2 * p + 2, qq * QW:(qq + 1) * QW],
                        start=(p == 0),
                        stop=(p == 1),
                        perf_mode=DR,
                    )
            nc.scalar.activation(
                out=exps[qq][:, 2 * mp:2 * mp + 2, :],
                in_=st,
                func=mybir.ActivationFunctionType.Exp,
                scale=float(scale),
                bias=bias_t,
            )

        # S(q0) woven with the remaining head transposes, then free the
        # head's PSUM banks for the AV/XT/C pools
        for mp in range(NB // 4):
            emit_s(0, mp)
        if head_tail is not None:
            head_tail()
        for mp in range(NB // 4, NB // 2):
            emit_s(0, mp)
        if head_tail2 is not None:
            head_tail2()
        if close_a is not None:
            close_a()
        psum_av = inner.enter_context(
            tc.tile_pool(name=f"psum_av{rep}", bufs=1, space="PSUM"))
        psum_xt = inner.enter_context(
            tc.tile_pool(name=f"psum_xt{rep}", bufs=1, space="PSUM"))
        psum_c = inner.enter_context(
            tc.tile_pool(name=f"psum_c{rep}", bufs=1, space="PSUM"))

        if True:
            def emit_a(n):
                qq, cl = n // NQT, (n % NQT) * P
                av = psum_av.tile([P, 513], f32, tag="av", name="av")
                for p in range(8):
                    lhsT = exps[qq][:, 2 * p:2 * p + 2, cl:cl + P]
                    nc.tensor.matmul(
                        av[:, 512:513],
                        lhsT,
                        vhat[:, 2 * p:2 * p + 2, 512:513],
                        start=(p == 0), stop=(p == 7),
                        perf_mode=DR,
                    )
                    nc.tensor.matmul(
                        av[:, 0:512],
                        lhsT,
                        vhat[:, 2 * p:2 * p + 2, 0:512],
                        start=(p == 0), stop=(p == 7),
                        perf_mode=DR,
                    )
                recip = ph3b.tile([P, 1], f32, tag="recip", name="recip")
                nc.vector.reciprocal(recip, av[:, 512:513])
                # r = relu(av/denom) on ScalarE (relu shares exp's act table,
                # so no table swap; recip > 0 lets relu commute with scaling)
                r_t = ph3.tile([P, D], bf16, tag="r", name="r_t")
                nc.vector.tensor_scalar(
                    out=r_t, in0=av[:, 0:512],
                    scalar1=recip, scalar2=0.0,
                    op0=mybir.AluOpType.mult, op1=mybir.AluOpType.max,
                )
                # x = V - R on GpSimd (SBUF-only operands)
                x_t = ph3b.tile([P, D], bf16, tag="x", name="x_t")
                x_eng = nc.vector if n >= NB - 1 else nc.gpsimd
                x_eng.tensor_tensor(
                    out=x_t, in0=vbf[:, n, :], in1=r_t,
                    op=mybir.AluOpType.subtract,
                )
                state[n] = (r_t, x_t)

            def emit_b(n):
                _, x_t = state[n]
                xt_ps = psum_xt.tile([P, DB, P], bf16, tag="xt", name="xt_ps")
                for j in range(DB):
                    nc.tensor.transpose(
                        xt_ps[:, j, :], x_t[:, j * P:(j + 1) * P], ident
                    )
                xt_sb = ph3b.tile([P, DB, P], bf16, tag="xts", name="xt_sb")
                # drain region: ACT is idle once exp ends, so its copy
                # parallels DVE's r/cb instead of queueing behind them
                if n >= NB - 3:
                    nc.scalar.copy(out=xt_sb, in_=xt_ps)
                else:
                    nc.vector.tensor_copy(out=xt_sb, in_=xt_ps)
                state[n] = (state[n][0], xt_sb)

            def emit_c(n):
                r_t, xt_sb = state.pop(n)
                if n >= NB - 2:
                    # the exp stream is done by the drain, so the last two
                    # C tiles borrow score-PSUM tiles: breaks the
                    # c(n)-waits-cb(n-1) serialization on the single c bank
                    c_ps = psum_st.tile([P, 2, QW], f32, tag="st",
                                        name="c15")[:, 0, :]
                else:
                    c_ps = psum_c.tile([P, D], f32, tag="c", name="c_ps")
                for ds in range(DB):
                    nc.tensor.matmul(
                        c_ps,
                        xt_sb[:, ds, :],
                        wt[:, ds, :],
                        start=(ds == 0),
                        stop=(ds == DB - 1),
                    )
                # cb = b * relu(C) on GpSimd (relu first, so any b is fine)
                cb_t = ph3b.tile([P, D], f32, tag="cb", name="cb_t")
                nc.vector.tensor_scalar(
                    out=cb_t, in0=c_ps,
                    scalar1=0.0, scalar2=float(b_val),
                    op0=mybir.AluOpType.max, op1=mybir.AluOpType.mult,
                )
                o_t = ph3.tile([P, D], f32, tag="o", name="o_t")
                o_eng = nc.vector if n >= NB - 1 else nc.gpsimd
                if a_val == 1.0:
                    o_eng.tensor_tensor(
                        out=o_t, in0=cb_t, in1=r_t, op=mybir.AluOpType.add
                    )
                else:
                    ra_t = ph3b.tile([P, D], f32, tag="ra", name="ra_t")
                    o_eng.tensor_scalar(
                        out=ra_t, in0=r_t,
                        scalar1=float(a_val), scalar2=None,
                        op0=mybir.AluOpType.mult,
                    )
                    o_eng.tensor_tensor(
                        out=o_t, in0=cb_t, in1=ra_t, op=mybir.AluOpType.add
                    )
                nc.sync.dma_start(out=out3[:, n, :], in_=o_t)

            LAGB, LAGC, NSW = 1, 2, 2
            spairs = [(c, mp) for c in (1, 2, 3) for mp in range(NB // 2)]
            for n in range(NB):
                if spairs:
                    emit_s(*spairs.pop(0))
                if qh1_piece is not None and n < 8:
                    qh1_piece(n, psum_xt)
                emit_a(n)
                for _ in range(NSW - 1):
                    if spairs:
                        emit_s(*spairs.pop(0))
                if n >= LAGB:
                    emit_b(n - LAGB)
                if n >= LAGC:
                    emit_c(n - LAGC)
            for n in range(NB - LAGB, NB):
                emit_b(n)
            for n in range(NB - LAGC, NB):
                emit_c(n)


def kernel(Q, K, V, W, scale_param, a, b):
    import sys
    if "/opt/trn_rl_repo" not in sys.path:
        sys.path.insert(0, "/opt/trn_rl_repo")
    from concourse.bass_utils import run_bass_kernel_spmd

    Q = np.ascontiguousarray(np.asarray(Q, dtype=np.float32))
    K = np.ascontiguousarray(np.asarray(K, dtype=np.float32))
    V = np.ascontiguousarray(np.asarray(V, dtype=np.float32))
    W = np.ascontiguousarray(np.asarray(W, dtype=np.float32))
    scale = float(np.asarray(scale_param).reshape(-1)[0]) / math.sqrt(D_FULL)
    a_val = float(np.asarray(a).reshape(-1)[0])
    b_val = float(np.asarray(b).reshape(-1)[0])

    nc = _build_bass(N_FULL, D_FULL, scale, a_val, b_val)
    in_maps = [
        {"q": Q[i], "k": K[i], "v": V[i], "w": W} for i in range(B)
    ]
    res = run_bass_kernel_spmd(nc, in_maps, list(range(B)))
    global LAST_RUN
    LAST_RUN = res
    out = np.stack([res.results[i]["out"] for i in range(B)])
    return out.astype(np.float32)


LAST_RUN = None

